# revision 21
# baseline (speedup 1.0000x reference)
"""Causal attention (with faithful missing-head-transpose reshape bug) on 8 Trainium2 cores.

Problem: B=2, T=2048, E=1024, H=16, dk=64.
  qkv = x @ w_qkv.T ; q,k,v split; per-head causal softmax attention;
  out = att_out[B,H,T,dk].reshape(B,T,E)  (NO head transpose — faithful bug);
  y = out @ w_proj.T + b_proj

Key observation: because of the missing transpose, output rows
y[b, 128h : 128h+128, :] depend ONLY on head h.  Sharding (batch x head-group)
over 8 cores therefore needs NO collectives: core c handles batch c//4 and
heads 4*(c%4) .. 4*(c%4)+3, producing output rows [512g, 512g+512) of batch b.

Per-core kernel v2 (single fused pipeline; ACT-exp is the pacing engine):
  - x is DMA'd t-window-major (3 merged transfers per tensor) so window-0
    QKV runs ~16us in and attention starts right after; the remaining QKV
    matmul groups are 8-MM jobs pumped into the attention phase's PE slack,
    borrowing the scores PSUM ring; a zero-matmul chain pre-warms HAM
    through the initial DMA wait
  - scores computed TRANSPOSED (keys on partitions) as concurrent 64-row
    tile pairs; diagonal blocks trim their matmul N to the unmasked columns
  - exp on ScalarE over full [128,1024] PSUM tiles (stale cols are finite
    and masked away); causal mask via width-trimmed DVE multiplies
  - V carries a ones-column so PV row 64 is the softmax denominator;
    reciprocal row is spread across lanes for the DVE recip, then
    replicated to 64 partitions with gpsimd.partition_broadcast mid-stream
    (hidden under ACT), or with K=1 PE outer products into the freed pv
    banks at the tail
  - output projection batched across all 4 heads (M = 4 heads x 32 rows
    per i-window, single-dim-split lhsT view) and run at the tail; a
    moving-operand fence matmul + explicit ldweights gate each window's
    projection so the PE reorder window cannot prefetch att2 before the
    norm muls and shift DMAs land (bridged through DVE touch copies)
  - bias applied on the DVE during PSUM evacuation (partition-broadcast
    b_proj); y stored with plain 2D DMAs in (w, h, r) row order and
    reordered on the host — 3D-rearranged DRAM scatter stores proved
    broken/flaky at the DMA level
"""

import os
import sys

import numpy as np

for _p in ("/opt/trn_rl_repo", "/root/.axon_site/_ro/trn_rl_repo"):
    if os.path.isdir(_p) and _p not in sys.path:
        sys.path.insert(0, _p)

import ml_dtypes  # noqa: E402

import concourse.bacc as bacc  # noqa: E402
import concourse.mybir as mybir  # noqa: E402
from concourse.bass import ds, ts  # noqa: E402
from concourse.tile import TileContext  # noqa: E402

F32 = mybir.dt.float32
BF16 = mybir.dt.bfloat16
AF = mybir.ActivationFunctionType
ALU = mybir.AluOpType
BF16NP = ml_dtypes.bfloat16

P = 128
E = 1024
DK = 64
HPC = 4  # heads per core
TW = 512  # i-window for scores / pv matmuls
EC = E // P  # 8 e-chunks
DC = (HPC * DK) // P  # 2 chunks of per-core qk features
FW = E // 512  # 2 output-feature windows


def build_nc(T=2048):
    W = T // TW  # i-windows (4)
    JPW = TW // P  # j-chunks per window (4)
    RR = (T * DK) // E  # rows of R per head (128)
    TT = E // DK  # 16 t-positions per R row
    RW = RR // W  # R-rows per i-window per head (32)
    NSP = HPC * TW // P  # denom elems per lane after spread (16)

    nc = bacc.Bacc("TRN2", target_bir_lowering=False, debug=False)
    xT = nc.declare_dram_parameter("xT", [E, T], BF16, isOutput=False)
    # q|k|v feature blocks packed side by side: 1.5KB DMA lines (vs 512B)
    wqkvT = nc.declare_dram_parameter("wqkvT", [E, 3 * HPC * DK], BF16, isOutput=False)
    wpT = nc.declare_dram_parameter("wpT", [E, E], BF16, isOutput=False)
    bp = nc.declare_dram_parameter("bp", [1, E], BF16, isOutput=False)
    y = nc.declare_dram_parameter("y", [HPC * RR, E], F32, isOutput=True)

    with nc.allow_low_precision(reason="bf16 matmuls; accumulation stays fp32 in PSUM"), TileContext(nc) as tc:
        with (
            tc.tile_pool(name="const", bufs=1) as const,
            tc.tile_pool(name="qkvout", bufs=1) as qkv_pool,
            tc.tile_pool(name="wp", bufs=1) as wp_pool,
            tc.tile_pool(name="att", bufs=1) as att_pool,
        ):
            # ---------------- sbuf layout (allocation only) ----------------
            dummy = const.tile([1, 8], F32)
            zer = const.tile([P, P], BF16)
            ones = const.tile([1, P], BF16, name="ones", tag="ones")
            # fence bridge: DVE touches copy one row of each att2 write-region
            # into fdum; the proj fence matmul reads fdum. DVE-after-DMA and
            # PE-after-DVE deps are reliable; PE-after-DMA proved flaky.
            fdum = const.tile([P, HPC * TW], BF16, name="fdum", tag="fdum")
            wsrc = const.tile([P, TW], BF16)
            masks = [
                const.tile([P, TW], BF16, name=f"mask{q}", tag=f"mask{q}")
                for q in range(JPW)
            ]
            bp_sb = const.tile([1, E], BF16)
            bp_b = const.tile([P, E], BF16, name="bp_b", tag="bp_b")

            wp_sb = wp_pool.tile([P, EC, E], BF16)
            wqkv_sb = qkv_pool.tile([P, EC, 3 * HPC * DK], BF16)
            # x split per i-window for hazard precision except w2+w3 merged
            # (2KB DMA lines); q, k, v per window
            xp01 = [qkv_pool.tile([P, EC, TW], BF16, name=f"xp{w}") for w in range(2)]
            xp23 = qkv_pool.tile([P, EC, 2 * TW], BF16, name="xp23")
            xpw = [xp01[0], xp01[1], xp23[:, :, 0:TW], xp23[:, :, TW : 2 * TW]]
            qTw = [qkv_pool.tile([P, DC, TW], BF16, name=f"qT{w}") for w in range(W)]
            kTw = [qkv_pool.tile([P, DC, TW], BF16, name=f"kT{w}") for w in range(W)]
            vsbw = [
                qkv_pool.tile([P, JPW, HPC * (DK + 1)], BF16, name=f"vsb{w}")
                for w in range(W)
            ]

            # all 4 heads' attention rows in ONE tile, column blocks ordered
            # (window, head) so the batched proj lhsT (h, r) slice merges into
            # a single free dim (stationary matmul APs allow only one)
            att2 = att_pool.tile([P, W, HPC * TW], BF16, name="att2", tag="att2")

            # ---------------- input DMA staging ----------------
            # per-queue DMA tops out ~100GB/s with these 1-1.5KB lines, so
            # stage0 (wqkv + x0) fans out over ALL THREE trigger queues in
            # 2-e-chunk transfers, wqkv/x alternating, so the first QKV matmul
            # group's e=0 deps are the FIRST transfer on their queues.
            engs0 = [nc.sync, nc.gpsimd, nc.scalar]
            engs = [nc.sync, nc.gpsimd, nc.scalar]
            qi = 0

            def dma0(out, in_):
                nonlocal qi
                engs0[qi % len(engs0)].dma_start(out=out, in_=in_)
                qi += 1

            def xchunk(dst, e0, ne, cols):
                dma0(
                    dst[:, e0 : e0 + ne, :],
                    xT[ds(P * e0, P * ne), cols].rearrange("(e p) t -> p e t", p=P),
                )

            for e0 in range(0, EC, 2):
                dma0(
                    wqkv_sb[:, e0 : e0 + 2, :],
                    wqkvT[ds(P * e0, 2 * P), :].rearrange("(e p) t -> p e t", p=P),
                )
                xchunk(xpw[0], e0, 2, slice(0, TW))
            # stage1: x1 (window-0 pumps need it ~25us in), then x2+x3 merged
            for e0, ne in [(0, 3), (3, 3), (6, 2)]:
                xchunk(xpw[1], e0, ne, slice(TW, 2 * TW))
            nc.sync.dma_start(
                out=xp23[:, 0:4, :],
                in_=xT[ds(0, 4 * P), 2 * TW : 4 * TW].rearrange("(e p) t -> p e t", p=P),
            )
            nc.gpsimd.dma_start(
                out=xp23[:, 4:8, :],
                in_=xT[ds(4 * P, 4 * P), 2 * TW : 4 * TW].rearrange("(e p) t -> p e t", p=P),
            )
            # stage2: w_proj + bias — keep off the scalar queue (it must stay
            # clear for the exp ACTIVATE stream)
            for e0, ne in [(0, 3), (3, 3), (6, 2)]:
                [nc.sync, nc.gpsimd, nc.sync][e0 // 3].dma_start(
                    out=wp_sb[:, e0 : e0 + ne, :],
                    in_=wpT[ds(P * e0, P * ne), :].rearrange("(e p) t -> p e t", p=P),
                )
            nc.gpsimd.dma_start(out=bp_sb, in_=bp[:, :])

            # ---------------- constants (emitted behind the triggers) ----------------
            nc.vector.memset(dummy, 0.0)
            nc.scalar.activation(dummy, dummy, AF.Exp, scale=1.0)  # preload exp table
            nc.vector.memset(zer, 0.0)
            nc.vector.memset(ones, 1.0)
            nc.vector.memset(wsrc, 0.0)
            # causal masks for the 4 diagonal-block offsets: keep j <= i - 128*q
            for q in range(JPW):
                mk = masks[q]
                nc.vector.memset(mk, 1.0)
                nc.gpsimd.affine_select(
                    out=mk,
                    in_=mk,
                    pattern=[[1, TW]],
                    compare_op=mybir.AluOpType.is_ge,
                    fill=0.0,
                    base=-P * q,
                    channel_multiplier=-1,
                )

            with (
                tc.tile_pool(name="exps", bufs=8) as epool,
                tc.tile_pool(name="rec", bufs=2) as rpool,
                tc.tile_pool(name="yout", bufs=2) as ypool,
                tc.tile_pool(name="psa", bufs=1, space="PSUM") as psa,
            ):
                # ones columns first: tiny, and the V evacuations need them.
                # (es ring needs NO pre-zero: exp writes and PV/mask reads are
                # both trimmed to the causally-live column range.)
                for w in range(W):
                    nc.gpsimd.memset(
                        vsbw[w].rearrange("p t (h c) -> p t h c", c=DK + 1)[
                            :, :, :, DK : DK + 1
                        ],
                        1.0,
                    )
                nc.gpsimd.memset(fdum, 0.0)
                # last col of each shifted att2 block is never written
                nc.gpsimd.memset(
                    att2.rearrange("p w (h t) -> p w h t", t=TW)[
                        DK : 2 * DK, :, :, TW - 1 : TW
                    ],
                    0.0,
                )
                # emitted last on gpsimd: blocks on the bp DMA (the final input
                # transfer) and is needed only by the tail projections
                nc.gpsimd.partition_broadcast(out_ap=bp_b[:, :], in_ap=bp_sb[0:1, :])

                # ---------------- QKV matmul group emitters ----------------
                # 8-MM jobs: short "s"-slot holds limit the ACT-pipeline squeeze
                QB, KB, VB = 0, HPC * DK, 2 * HPC * DK  # col bases in wqkv_sb

                def emit_qk_half(dstw, base, w, dc):
                    st = psa.tile([P, 2 * TW], F32, tag="s", bufs=2, name="st_qk")
                    for e in range(EC):
                        nc.tensor.matmul(
                            st[:, 0:TW],
                            wqkv_sb[:, e, ds(base + P * dc, P)],
                            xpw[w][:, e, :],
                            start=(e == 0),
                            stop=(e == EC - 1),
                        )
                    nc.vector.tensor_copy(dstw[w][:, dc, :], st[:, 0:TW])

                def emit_v_one(w, i0):
                    # one V t-chunk of window w
                    st = psa.tile([P, 2 * TW], F32, tag="s", bufs=2, name="st_v")
                    for e in range(EC):
                        nc.tensor.matmul(
                            st[:, 0 : HPC * DK],
                            xpw[w][:, e, ts(i0, P)],
                            wqkv_sb[:, e, VB : VB + HPC * DK],
                            start=(e == 0),
                            stop=(e == EC - 1),
                        )
                    nc.vector.tensor_copy(
                        vsbw[w].rearrange("p t (h c) -> p t h c", c=DK + 1)[
                            :, i0, :, 0:DK
                        ],
                        st[:, 0 : HPC * DK].rearrange("p (h d) -> p h d", d=DK),
                    )

                def queue_qkv(w):
                    # q first (needed by the next window's first scores), then
                    # k halves, then the four V chunks
                    for dc in range(DC):
                        side_q.append(lambda dc=dc: emit_qk_half(qTw, QB, w, dc))
                    for dc in range(DC):
                        side_q.append(lambda dc=dc: emit_qk_half(kTw, KB, w, dc))
                    for i0 in range(JPW):
                        side_q.append(lambda i0=i0: emit_v_one(w, i0))

                side_q = []  # deferred work: QKV groups, norm chains
                projq = []  # all projections run at the tail (LDW-prefetch safety)

                def pump():
                    if side_q:
                        side_q.pop(0)()

                # HAM pre-warm during the input DMA wait (borrows the pv0 bank
                # long before the first real PV accumulation); sized to end
                # right when the first wqkv/x0 chunks land (~13us) so the PE
                # never cools into the throttled-ramp state before real work
                warm = psa.tile([P, TW], F32, tag="pv0", bufs=1, name="warm")
                NWARM0 = 44
                for i in range(NWARM0):
                    nc.tensor.matmul(
                        warm[0 : DK + 1, 0:P],
                        zer[:, 0 : DK + 1],
                        wsrc[:, 0:P],
                        start=(i == 0),
                        stop=(i == NWARM0 - 1),
                    )

                # window-0 K/Q up front; V chunks interleaved with window 1's
                # Q/K halves so both meet their deadlines within w0's 8 pumps
                for dc in range(DC):
                    emit_qk_half(kTw, KB, 0, dc)
                for dc in range(DC):
                    emit_qk_half(qTw, QB, 0, dc)
                # V chunks 0-1 (PV jc0/jc1) inline; the rest via pumps
                emit_v_one(0, 0)
                emit_v_one(0, 1)
                side_q.append(lambda: emit_v_one(0, 2))
                side_q.append(lambda: emit_v_one(0, 3))
                side_q.append(lambda: emit_qk_half(qTw, QB, 1, 0))
                side_q.append(lambda: emit_qk_half(qTw, QB, 1, 1))
                side_q.append(lambda: emit_qk_half(kTw, KB, 1, 0))
                side_q.append(lambda: emit_qk_half(kTw, KB, 1, 1))

                # ---------------- attention ----------------
                def make_norm_chain(w, pvs, heads, drain_engs=None):
                    """Closures: drain pv banks, build reciprocal row, broadcast
                    to 64 partitions, normalize att2, shift-dup, then the batched
                    4-head projection for window w. `heads` is a contiguous
                    subset of local heads so the last window can pipeline its
                    two head pairs."""
                    nh = len(heads)
                    nsp = nh * TW // P  # denom elems per lane after spread
                    praws = []

                    def drain():
                        dn = rpool.tile([P, HPC * TW], F32, name="dn", tag="dn")
                        praws.append(dn)
                        for i, h in enumerate(heads):
                            eng = drain_engs[i] if drain_engs else nc.vector
                            praw = rpool.tile(
                                [P, TW], BF16, name="praw", tag=f"praw{h}", bufs=1
                            )
                            if eng is nc.scalar:
                                nc.scalar.copy(praw[0:DK, :], pvs[h][0:DK, :])
                                nc.scalar.copy(
                                    dn[DK : DK + 1, ds(TW * i, TW)],
                                    pvs[h][DK : DK + 1, :],
                                )
                            else:
                                eng.tensor_copy(praw[0:DK, :], pvs[h][0:DK, :])
                                eng.tensor_copy(
                                    dn[DK : DK + 1, ds(TW * i, TW)],
                                    pvs[h][DK : DK + 1, :],
                                )
                            praws.append(praw)

                    def recip():
                        dn = praws[0]
                        sp = rpool.tile([P, 2 * NSP], F32, name="sp", tag="sp")
                        nc.sync.dma_start(
                            out=sp[:, 0:nsp],
                            in_=dn[DK : DK + 1, 0 : nh * TW].rearrange(
                                "a (p c) -> a p c", c=nsp
                            ),
                        )
                        nc.vector.reciprocal(
                            out=sp[:, NSP : NSP + nsp], in_=sp[:, 0:nsp]
                        )
                        spb = rpool.tile([P, NSP], BF16, name="spb", tag="spb")
                        nc.vector.tensor_copy(
                            spb[:, 0:nsp], sp[:, NSP : NSP + nsp]
                        )
                        rb = rpool.tile([P, HPC * TW], BF16, name="rb", tag="rb")
                        nc.sync.dma_start(
                            out=rb[0:1, 0 : nh * TW].rearrange(
                                "a (p c) -> a p c", c=nsp
                            ),
                            in_=spb[:, 0:nsp],
                        )
                        rb2 = rpool.tile([P, HPC * TW], BF16, name="rb2", tag="rb2")
                        nc.gpsimd.partition_broadcast(
                            out_ap=rb2[0:DK, 0 : nh * TW], in_ap=rb[0:1, 0 : nh * TW]
                        )
                        praws.append(rb2)

                    def norm2():
                        rb2 = praws[nh + 1]
                        for i, h in enumerate(heads):
                            nc.vector.tensor_mul(
                                att2[0:DK, w, ds(TW * h, TW)],
                                praws[1 + i][0:DK, :],
                                rb2[0:DK, ds(TW * i, TW)],
                            )
                            # shifted-dup half: proj only reads its even local
                            # cols (<= TW-2), so the shift is block-local
                            nc.sync.dma_start(
                                out=att2[DK : 2 * DK, w, TW * h : TW * h + TW - 1],
                                in_=att2[0:DK, w, TW * h + 1 : TW * h + TW],
                            )
                            # fence-bridge touches (see fdum above): tiny per
                            # (window, head) reads/writes in fdum's window-w
                            # region, so proj(w)'s fence read overlaps exactly
                            # its own window's touches and no other's (the RAW
                            # dep is instruction-granular: an 8-col read of the
                            # mul/shift outputs carries the full dependency)
                            nc.vector.tensor_copy(
                                fdum[0:1, ds(TW * w + 16 * h, 8)],
                                att2[0:1, w, ds(TW * h, 8)],
                            )
                            nc.vector.tensor_copy(
                                fdum[DK : DK + 1, ds(TW * w + 16 * h, 8)],
                                att2[DK : DK + 1, w, ds(TW * h, 8)],
                            )

                    # single-dim split view (baseline-proven pattern): col
                    # index c = 32h + r merges (h, r) naturally in memory
                    a2v = att2.rearrange("p w (c t) -> p w c t", t=TT)

                    def proj(fw):
                        def go():
                            yp = psa.tile([P, 2 * TW], F32, tag="s", bufs=2, name="yp")
                            if fw == 0:
                                # fence 1: moving-operand read of fdum's
                                # window-w region (written by the DVE touches
                                # after this window's muls and shift DMAs)
                                # stalls the PE dispatch.
                                nc.tensor.matmul(
                                    yp[0:1, 0:DK],
                                    zer[:, 0:1],
                                    fdum[:, ds(TW * w, DK)],
                                    start=True,
                                    stop=True,
                                )
                                # fence 2: an explicit LDWEIGHTS reading att2
                                # carries the RAW wait on the weight-load
                                # itself — weight-loads execute in order, so
                                # the PE reorder window cannot prefetch the
                                # proj lhsT past this waiting load.
                                nc.tensor.ldweights(
                                    weights=a2v[:, w, :, 0:1]
                                )
                            for m in range(EC):
                                nc.tensor.matmul(
                                    yp[:, 0:TW],
                                    a2v[:, w, :, 2 * m : 2 * m + 1],
                                    wp_sb[:, m, ds(TW * fw, TW)],
                                    start=(m == 0),
                                    stop=(m == EC - 1),
                                )
                            ysb = ypool.tile([P, TW], F32, name="ysb")
                            nc.vector.scalar_tensor_tensor(
                                out=ysb,
                                in0=yp[:, 0:TW],
                                scalar=1.0,
                                in1=bp_b[:, ds(TW * fw, TW)],
                                op0=ALU.mult,
                                op1=ALU.add,
                            )
                            # simple 2D store in (w, h, r) row order; the host
                            # reorders to (h, w, r)
                            engs[(2 * w + fw) % 2].dma_start(
                                out=y[ds(P * w, P), ds(TW * fw, TW)],
                                in_=ysb,
                            )

                        return go

                    return [drain, recip, norm2, proj(0), proj(1)]

                def emit_pv(pvs, ess, jc, njc, trim):
                    # trim: on diagonal blocks, q-cols < 128*qq are fully
                    # masked (es there is stale/garbage) — skip them. Safe:
                    # every pv element's FIRST writer is the full-width jc==0
                    # block (qq<=0 there), so start-flag coverage is complete.
                    wj, jj = divmod(jc, JPW)
                    for es, p in ess:
                        for sub in range(2):
                            h = 2 * p + sub
                            nc.tensor.matmul(
                                pvs[h][0 : DK + 1, trim:TW],
                                vsbw[wj][:, jj, ds((DK + 1) * h, DK + 1)],
                                es[:, TW * sub + trim : TW * (sub + 1)],
                                start=(jc == 0),
                                stop=(jc == njc - 1),
                            )

                def attn_phase(w, pvs, ps):
                    # scores -> exp -> mask -> pv for head-pairs in ps, over
                    # all causally-live j-blocks of window w
                    njc = JPW * (w + 1)
                    pend = []
                    for jc in range(njc):
                        qq = jc - JPW * w  # >=0 on causal-diagonal blocks
                        trim = P * qq if qq > 0 else 0
                        ess = []
                        for p in ps:
                            st = psa.tile([P, 2 * TW], F32, tag="s", bufs=2, name="st")
                            for sub in range(2):
                                nc.tensor.matmul(
                                    st[:, TW * sub + trim : TW * (sub + 1)],
                                    kTw[jc // JPW][
                                        ds(DK * sub, DK), p, ds(P * (jc % JPW), P)
                                    ],
                                    qTw[w][ds(DK * sub, DK), p, trim:TW],
                                    start=True,
                                    stop=True,
                                )
                            es = epool.tile([P, 2 * TW], BF16, name="es")
                            if trim:
                                # strided view skips the fully-masked q-cols of
                                # both subs in ONE ACTIVATE (ACT is co-critical)
                                nc.scalar.activation(
                                    es.rearrange("p (s t) -> p s t", s=2)[
                                        :, :, trim:TW
                                    ],
                                    st.rearrange("p (s t) -> p s t", s=2)[
                                        :, :, trim:TW
                                    ],
                                    AF.Exp,
                                    scale=1.0 / 8.0,
                                )
                            else:
                                nc.scalar.activation(es, st, AF.Exp, scale=1.0 / 8.0)
                            if qq >= 0:
                                # mask-mul on gpsimd: keeps the DVE queue clear
                                # for qkv evacuations (PE-gating) — gpsimd is
                                # otherwise idle mid-body
                                mw = P * (qq + 1)  # mask strip end (rest keep-all)
                                for sub in range(2):
                                    nc.gpsimd.tensor_mul(
                                        es[:, TW * sub + trim : TW * sub + mw],
                                        es[:, TW * sub + trim : TW * sub + mw],
                                        masks[qq][:, trim:mw],
                                    )
                            ess.append((es, p))
                        pend.append((ess, jc, trim))
                        pump()
                        if len(pend) > 2:
                            e0 = pend.pop(0)
                            emit_pv(pvs, e0[0], e0[1], njc, e0[2])
                            pump()
                    for e0 in pend:
                        emit_pv(pvs, e0[0], e0[1], njc, e0[2])
                        pump()

                for w in range(W):
                    # queue next window's QKV groups (w0's were interleaved above)
                    if 0 < w < W - 1:
                        queue_qkv(w + 1)
                    elif w == 0:
                        for i0 in range(JPW):
                            side_q.append(lambda i0=i0: emit_v_one(1, i0))

                    pvs = [
                        psa.tile([P, TW], F32, tag=f"pv{h}", bufs=1, name=f"pv{h}")
                        for h in range(HPC)
                    ]
                    if w + 1 < W:
                        attn_phase(w, pvs, [0, 1])
                        # drain pv banks NOW (before the next window's pv ring
                        # allocation — the ring's WAR tracking only sees
                        # readers already emitted); the rest is deferred
                        chain = make_norm_chain(w, pvs, [0, 1, 2, 3])
                        chain[0]()
                        side_q.extend(chain[1:3])
                        projq.extend(chain[3:])
                    else:
                        # final window, split by head pair: heads 0/1's norm
                        # chain flows on DVE/DMA/gpsimd while heads 2/3's
                        # blocks keep the PE busy; the remaining h2/h3 chain
                        # then hides under windows 0-2's deferred projections.
                        attn_phase(w, pvs, [0])
                        chainA = make_norm_chain(w, pvs, [0, 1])
                        chainA[0]()
                        side_q.extend(chainA[1:3])
                        attn_phase(w, pvs, [1])
                        chainB = make_norm_chain(
                            w, pvs, [2, 3], drain_engs=[nc.vector, nc.scalar]
                        )
                        chainB[0]()
                        chainB[1]()
                        chainB[2]()
                        for pj in projq:
                            pj()
                        chainB[3]()
                        chainB[4]()
    nc.compile()
    return nc


_CACHE = {}
LAST_RESULT = None


def _get_nc(T=2048):
    key = ("nc", T)
    if key not in _CACHE:
        _CACHE[key] = build_nc(T=T)
    return _CACHE[key]


def make_in_maps(x, w_qkv, w_proj, b_proj):
    B, T, _E = x.shape
    in_maps = []
    wpTh = np.ascontiguousarray(w_proj.T.astype(BF16NP))
    bph = np.ascontiguousarray(b_proj.reshape(1, E).astype(BF16NP))
    xTs = [np.ascontiguousarray(x[b].T.astype(BF16NP)) for b in range(B)]
    for c in range(8):
        b, g = divmod(c, 4)
        r0 = HPC * DK * g  # 256*g
        sl = slice(r0, r0 + HPC * DK)
        wqkvTh = np.ascontiguousarray(
            np.concatenate(
                [
                    w_qkv[sl, :].T,
                    w_qkv[E:][sl, :].T,
                    w_qkv[2 * E :][sl, :].T,
                ],
                axis=1,
            ).astype(BF16NP)
        )
        in_maps.append(
            {
                "xT": xTs[b],
                "wqkvT": wqkvTh,
                "wpT": wpTh,
                "bp": bph,
            }
        )
    return in_maps


def kernel(x, w_qkv, w_proj, b_proj):
    global LAST_RESULT
    from concourse.bass_utils import run_bass_kernel_spmd

    x = np.asarray(x, dtype=np.float32)
    w_qkv = np.asarray(w_qkv, dtype=np.float32)
    w_proj = np.asarray(w_proj, dtype=np.float32)
    b_proj = np.asarray(b_proj, dtype=np.float32)
    B, T, _E = x.shape

    nc = _get_nc(T=T)
    in_maps = make_in_maps(x, w_qkv, w_proj, b_proj)
    res = run_bass_kernel_spmd(nc, in_maps, core_ids=list(range(8)))
    LAST_RESULT = res

    out = np.empty((B, T, E), dtype=np.float32)
    rows = HPC * ((T * DK) // E)  # 512 rows per core
    W = T // 512
    RW = ((T * DK) // E) // W
    for c in range(8):
        b, g = divmod(c, 4)
        yc = res.results[c]["y"].reshape(W, HPC, RW, E).transpose(1, 0, 2, 3)
        out[b, rows * g : rows * (g + 1), :] = yc.reshape(rows, E)
    return out



# revision 22
# speedup vs baseline: 1.3120x; 1.3120x over previous
"""Causal attention (with faithful missing-head-transpose reshape bug) on 8 Trainium2 cores.

Problem: B=2, T=2048, E=1024, H=16, dk=64.
  qkv = x @ w_qkv.T ; q,k,v split; per-head causal softmax attention;
  out = att_out[B,H,T,dk].reshape(B,T,E)  (NO head transpose — faithful bug);
  y = out @ w_proj.T + b_proj

Key observation: because of the missing transpose, output rows
y[b, 128h : 128h+128, :] depend ONLY on head h.  Sharding (batch x head-group)
over 8 cores therefore needs NO collectives: core c handles batch c//4 and
heads 4*(c%4) .. 4*(c%4)+3, producing output rows [512g, 512g+512) of batch b.

Per-core kernel v2 (single fused pipeline; ACT-exp is the pacing engine):
  - x is DMA'd t-window-major (3 merged transfers per tensor) so window-0
    QKV runs ~16us in and attention starts right after; the remaining QKV
    matmul groups are 8-MM jobs pumped into the attention phase's PE slack,
    borrowing the scores PSUM ring; a zero-matmul chain pre-warms HAM
    through the initial DMA wait
  - scores computed TRANSPOSED (keys on partitions) as concurrent 64-row
    tile pairs; diagonal blocks trim their matmul N to the unmasked columns
  - exp on ScalarE over full [128,1024] PSUM tiles (stale cols are finite
    and masked away); causal mask via width-trimmed DVE multiplies
  - V carries a ones-column so PV row 64 is the softmax denominator;
    reciprocal row is spread across lanes for the DVE recip, then
    replicated to 64 partitions with gpsimd.partition_broadcast mid-stream
    (hidden under ACT), or with K=1 PE outer products into the freed pv
    banks at the tail
  - output projection batched across all 4 heads (M = 4 heads x 32 rows
    per i-window, single-dim-split lhsT view) and run at the tail; a
    moving-operand fence matmul + explicit ldweights gate each window's
    projection so the PE reorder window cannot prefetch att2 before the
    norm muls and shift DMAs land (bridged through DVE touch copies)
  - bias applied on the DVE during PSUM evacuation (partition-broadcast
    b_proj); y stored with plain 2D DMAs in (w, h, r) row order and
    reordered on the host — 3D-rearranged DRAM scatter stores proved
    broken/flaky at the DMA level
"""

import os
import sys

import numpy as np

for _p in ("/opt/trn_rl_repo", "/root/.axon_site/_ro/trn_rl_repo"):
    if os.path.isdir(_p) and _p not in sys.path:
        sys.path.insert(0, _p)

import ml_dtypes  # noqa: E402

import concourse.bacc as bacc  # noqa: E402
import concourse.mybir as mybir  # noqa: E402
from concourse.bass import ds, ts  # noqa: E402
from concourse.tile import TileContext  # noqa: E402

F32 = mybir.dt.float32
BF16 = mybir.dt.bfloat16
AF = mybir.ActivationFunctionType
ALU = mybir.AluOpType
BF16NP = ml_dtypes.bfloat16

P = 128
E = 1024
DK = 64
HPC = 4  # heads per core
TW = 512  # i-window for scores / pv matmuls
EC = E // P  # 8 e-chunks
DC = (HPC * DK) // P  # 2 chunks of per-core qk features
FW = E // 512  # 2 output-feature windows


def build_nc(T=2048):
    W = T // TW  # i-windows (4)
    JPW = TW // P  # j-chunks per window (4)
    RR = (T * DK) // E  # rows of R per head (128)
    TT = E // DK  # 16 t-positions per R row
    RW = RR // W  # R-rows per i-window per head (32)
    NSP = HPC * TW // P  # denom elems per lane after spread (16)

    nc = bacc.Bacc("TRN2", target_bir_lowering=False, debug=False)
    xT = nc.declare_dram_parameter("xT", [E, T], BF16, isOutput=False)
    # q|k|v feature blocks packed side by side: 1.5KB DMA lines (vs 512B)
    wqkvT = nc.declare_dram_parameter("wqkvT", [E, 3 * HPC * DK], BF16, isOutput=False)
    wpT = nc.declare_dram_parameter("wpT", [E, E], BF16, isOutput=False)
    bp = nc.declare_dram_parameter("bp", [1, E], BF16, isOutput=False)
    y = nc.declare_dram_parameter("y", [HPC * RR, E], F32, isOutput=True)

    with nc.allow_low_precision(reason="bf16 matmuls; accumulation stays fp32 in PSUM"), TileContext(nc) as tc:
        with (
            tc.tile_pool(name="const", bufs=1) as const,
            tc.tile_pool(name="qkvout", bufs=1) as qkv_pool,
            tc.tile_pool(name="wp", bufs=1) as wp_pool,
            tc.tile_pool(name="att", bufs=1) as att_pool,
        ):
            # ---------------- sbuf layout (allocation only) ----------------
            dummy = const.tile([1, 8], F32)
            zer = const.tile([P, P], BF16)
            ones = const.tile([1, P], BF16, name="ones", tag="ones")
            # fence bridge: DVE touches copy one row of each att2 write-region
            # into fdum; the proj fence matmul reads fdum. DVE-after-DMA and
            # PE-after-DVE deps are reliable; PE-after-DMA proved flaky.
            fdum = const.tile([P, HPC * TW], BF16, name="fdum", tag="fdum")
            wsrc = const.tile([P, TW], BF16)
            masks = [
                const.tile([P, TW], BF16, name=f"mask{q}", tag=f"mask{q}")
                for q in range(JPW)
            ]
            bp_sb = const.tile([1, E], BF16)
            bp_b = const.tile([P, E], BF16, name="bp_b", tag="bp_b")

            wp_sb = wp_pool.tile([P, EC, E], BF16)
            wqkv_sb = qkv_pool.tile([P, EC, 3 * HPC * DK], BF16)
            # x split per i-window for hazard precision except w2+w3 merged
            # (2KB DMA lines); q, k, v per window
            xp01 = [qkv_pool.tile([P, EC, TW], BF16, name=f"xp{w}") for w in range(2)]
            xp23 = qkv_pool.tile([P, EC, 2 * TW], BF16, name="xp23")
            xpw = [xp01[0], xp01[1], xp23[:, :, 0:TW], xp23[:, :, TW : 2 * TW]]
            qTw = [qkv_pool.tile([P, DC, TW], BF16, name=f"qT{w}") for w in range(W)]
            kTw = [qkv_pool.tile([P, DC, TW], BF16, name=f"kT{w}") for w in range(W)]
            vsbw = [
                qkv_pool.tile([P, JPW, HPC * (DK + 1)], BF16, name=f"vsb{w}")
                for w in range(W)
            ]

            # all 4 heads' attention rows in ONE tile, column blocks ordered
            # (window, head) so the batched proj lhsT (h, r) slice merges into
            # a single free dim (stationary matmul APs allow only one)
            att2 = att_pool.tile([P, W, HPC * TW], BF16, name="att2", tag="att2")

            # ---------------- input DMA staging ----------------
            # per-queue DMA tops out ~100GB/s with these 1-1.5KB lines, so
            # stage0 (wqkv + x0) fans out over ALL THREE trigger queues in
            # 2-e-chunk transfers, wqkv/x alternating, so the first QKV matmul
            # group's e=0 deps are the FIRST transfer on their queues.
            engs0 = [nc.sync, nc.gpsimd, nc.scalar]
            engs = [nc.sync, nc.gpsimd, nc.scalar]
            qi = 0

            def dma0(out, in_):
                nonlocal qi
                engs0[qi % len(engs0)].dma_start(out=out, in_=in_)
                qi += 1

            def xchunk(dst, e0, ne, cols):
                dma0(
                    dst[:, e0 : e0 + ne, :],
                    xT[ds(P * e0, P * ne), cols].rearrange("(e p) t -> p e t", p=P),
                )

            for e0 in range(0, EC, 2):
                dma0(
                    wqkv_sb[:, e0 : e0 + 2, :],
                    wqkvT[ds(P * e0, 2 * P), :].rearrange("(e p) t -> p e t", p=P),
                )
                xchunk(xpw[0], e0, 2, slice(0, TW))
            # stage1: x1 (window-0 pumps need it ~25us in), then x2+x3 merged
            for e0, ne in [(0, 3), (3, 3), (6, 2)]:
                xchunk(xpw[1], e0, ne, slice(TW, 2 * TW))
            nc.sync.dma_start(
                out=xp23[:, 0:4, :],
                in_=xT[ds(0, 4 * P), 2 * TW : 4 * TW].rearrange("(e p) t -> p e t", p=P),
            )
            nc.gpsimd.dma_start(
                out=xp23[:, 4:8, :],
                in_=xT[ds(4 * P, 4 * P), 2 * TW : 4 * TW].rearrange("(e p) t -> p e t", p=P),
            )
            # stage2: w_proj + bias — keep off the scalar queue (it must stay
            # clear for the exp ACTIVATE stream)
            for e0, ne in [(0, 3), (3, 3), (6, 2)]:
                [nc.sync, nc.gpsimd, nc.sync][e0 // 3].dma_start(
                    out=wp_sb[:, e0 : e0 + ne, :],
                    in_=wpT[ds(P * e0, P * ne), :].rearrange("(e p) t -> p e t", p=P),
                )
            nc.gpsimd.dma_start(out=bp_sb, in_=bp[:, :])

            # ---------------- constants (emitted behind the triggers) ----------------
            nc.vector.memset(dummy, 0.0)
            nc.scalar.activation(dummy, dummy, AF.Exp, scale=1.0)  # preload exp table
            nc.vector.memset(zer, 0.0)
            nc.vector.memset(ones, 1.0)
            nc.vector.memset(wsrc, 0.0)
            # causal masks for the 4 diagonal-block offsets: keep j <= i - 128*q
            for q in range(JPW):
                mk = masks[q]
                nc.vector.memset(mk, 1.0)
                nc.gpsimd.affine_select(
                    out=mk,
                    in_=mk,
                    pattern=[[1, TW]],
                    compare_op=mybir.AluOpType.is_ge,
                    fill=0.0,
                    base=-P * q,
                    channel_multiplier=-1,
                )

            with (
                tc.tile_pool(name="exps", bufs=8) as epool,
                tc.tile_pool(name="rec", bufs=2) as rpool,
                tc.tile_pool(name="yout", bufs=2) as ypool,
                tc.tile_pool(name="psa", bufs=1, space="PSUM") as psa,
            ):
                # ones columns first: tiny, and the V evacuations need them.
                # (es ring needs NO pre-zero: exp writes and PV/mask reads are
                # both trimmed to the causally-live column range.)
                for w in range(W):
                    nc.gpsimd.memset(
                        vsbw[w].rearrange("p t (h c) -> p t h c", c=DK + 1)[
                            :, :, :, DK : DK + 1
                        ],
                        1.0,
                    )
                nc.gpsimd.memset(fdum, 0.0)
                # last col of each shifted att2 block is never written
                nc.gpsimd.memset(
                    att2.rearrange("p w (h t) -> p w h t", t=TW)[
                        DK : 2 * DK, :, :, TW - 1 : TW
                    ],
                    0.0,
                )
                # emitted last on gpsimd: blocks on the bp DMA (the final input
                # transfer) and is needed only by the tail projections
                nc.gpsimd.partition_broadcast(out_ap=bp_b[:, :], in_ap=bp_sb[0:1, :])

                # ---------------- QKV matmul group emitters ----------------
                # 8-MM jobs: short "s"-slot holds limit the ACT-pipeline squeeze
                QB, KB, VB = 0, HPC * DK, 2 * HPC * DK  # col bases in wqkv_sb

                def emit_qk_half(dstw, base, w, dc):
                    st = psa.tile([P, 2 * TW], F32, tag="s", bufs=2, name="st_qk")
                    for e in range(EC):
                        nc.tensor.matmul(
                            st[:, 0:TW],
                            wqkv_sb[:, e, ds(base + P * dc, P)],
                            xpw[w][:, e, :],
                            start=(e == 0),
                            stop=(e == EC - 1),
                        )
                    nc.vector.tensor_copy(dstw[w][:, dc, :], st[:, 0:TW])

                def emit_v_one(w, i0):
                    # one V t-chunk of window w
                    st = psa.tile([P, 2 * TW], F32, tag="s", bufs=2, name="st_v")
                    for e in range(EC):
                        nc.tensor.matmul(
                            st[:, 0 : HPC * DK],
                            xpw[w][:, e, ts(i0, P)],
                            wqkv_sb[:, e, VB : VB + HPC * DK],
                            start=(e == 0),
                            stop=(e == EC - 1),
                        )
                    nc.vector.tensor_copy(
                        vsbw[w].rearrange("p t (h c) -> p t h c", c=DK + 1)[
                            :, i0, :, 0:DK
                        ],
                        st[:, 0 : HPC * DK].rearrange("p (h d) -> p h d", d=DK),
                    )

                def queue_qkv(w):
                    # q first (needed by the next window's first scores), then
                    # k halves, then the four V chunks
                    for dc in range(DC):
                        side_q.append(lambda dc=dc: emit_qk_half(qTw, QB, w, dc))
                    for dc in range(DC):
                        side_q.append(lambda dc=dc: emit_qk_half(kTw, KB, w, dc))
                    for i0 in range(JPW):
                        side_q.append(lambda i0=i0: emit_v_one(w, i0))

                side_q = []  # deferred work: QKV groups, norm chains
                projq = []  # all projections run at the tail (LDW-prefetch safety)

                def pump():
                    if side_q:
                        side_q.pop(0)()

                # HAM pre-warm during the input DMA wait (borrows the pv0 bank
                # long before the first real PV accumulation); sized to end
                # right when the first wqkv/x0 chunks land (~13us) so the PE
                # never cools into the throttled-ramp state before real work
                warm = psa.tile([P, TW], F32, tag="pv0", bufs=1, name="warm")
                NWARM0 = 44
                for i in range(NWARM0):
                    nc.tensor.matmul(
                        warm[0 : DK + 1, 0:P],
                        zer[:, 0 : DK + 1],
                        wsrc[:, 0:P],
                        start=(i == 0),
                        stop=(i == NWARM0 - 1),
                    )

                # window-0 K/Q up front; V chunks interleaved with window 1's
                # Q/K halves so both meet their deadlines within w0's 8 pumps
                for dc in range(DC):
                    emit_qk_half(kTw, KB, 0, dc)
                for dc in range(DC):
                    emit_qk_half(qTw, QB, 0, dc)
                # V chunks 0-1 (PV jc0/jc1) inline; the rest via pumps
                emit_v_one(0, 0)
                emit_v_one(0, 1)
                side_q.append(lambda: emit_v_one(0, 2))
                side_q.append(lambda: emit_v_one(0, 3))
                side_q.append(lambda: emit_qk_half(qTw, QB, 1, 0))
                side_q.append(lambda: emit_qk_half(qTw, QB, 1, 1))
                side_q.append(lambda: emit_qk_half(kTw, KB, 1, 0))
                side_q.append(lambda: emit_qk_half(kTw, KB, 1, 1))

                # ---------------- attention ----------------
                def make_norm_chain(w, pvs, heads, drain_engs=None):
                    """Closures: drain pv banks, build reciprocal row, broadcast
                    to 64 partitions, normalize att2, shift-dup, then the batched
                    4-head projection for window w. `heads` is a contiguous
                    subset of local heads so the last window can pipeline its
                    two head pairs."""
                    nh = len(heads)
                    nsp = nh * TW // P  # denom elems per lane after spread
                    praws = []

                    def drain():
                        dn = rpool.tile([P, HPC * TW], F32, name="dn", tag="dn")
                        praws.append(dn)
                        for i, h in enumerate(heads):
                            eng = drain_engs[i] if drain_engs else nc.vector
                            praw = rpool.tile(
                                [P, TW], BF16, name="praw", tag=f"praw{h}", bufs=1
                            )
                            if eng is nc.scalar:
                                nc.scalar.copy(praw[0:DK, :], pvs[h][0:DK, :])
                                nc.scalar.copy(
                                    dn[DK : DK + 1, ds(TW * i, TW)],
                                    pvs[h][DK : DK + 1, :],
                                )
                            else:
                                eng.tensor_copy(praw[0:DK, :], pvs[h][0:DK, :])
                                eng.tensor_copy(
                                    dn[DK : DK + 1, ds(TW * i, TW)],
                                    pvs[h][DK : DK + 1, :],
                                )
                            praws.append(praw)

                    def recip():
                        dn = praws[0]
                        sp = rpool.tile([P, 2 * NSP], F32, name="sp", tag="sp")
                        nc.sync.dma_start(
                            out=sp[:, 0:nsp],
                            in_=dn[DK : DK + 1, 0 : nh * TW].rearrange(
                                "a (p c) -> a p c", c=nsp
                            ),
                        )
                        nc.vector.reciprocal(
                            out=sp[:, NSP : NSP + nsp], in_=sp[:, 0:nsp]
                        )
                        spb = rpool.tile([P, NSP], BF16, name="spb", tag="spb")
                        nc.vector.tensor_copy(
                            spb[:, 0:nsp], sp[:, NSP : NSP + nsp]
                        )
                        rb = rpool.tile([P, HPC * TW], BF16, name="rb", tag="rb")
                        nc.sync.dma_start(
                            out=rb[0:1, 0 : nh * TW].rearrange(
                                "a (p c) -> a p c", c=nsp
                            ),
                            in_=spb[:, 0:nsp],
                        )
                        rb2 = rpool.tile([P, HPC * TW], BF16, name="rb2", tag="rb2")
                        nc.gpsimd.partition_broadcast(
                            out_ap=rb2[0:DK, 0 : nh * TW], in_ap=rb[0:1, 0 : nh * TW]
                        )
                        praws.append(rb2)

                    def norm2():
                        rb2 = praws[nh + 1]
                        for i, h in enumerate(heads):
                            nc.vector.tensor_mul(
                                att2[0:DK, w, ds(TW * h, TW)],
                                praws[1 + i][0:DK, :],
                                rb2[0:DK, ds(TW * i, TW)],
                            )
                            # shifted-dup half: proj only reads its even local
                            # cols (<= TW-2), so the shift is block-local
                            nc.sync.dma_start(
                                out=att2[DK : 2 * DK, w, TW * h : TW * h + TW - 1],
                                in_=att2[0:DK, w, TW * h + 1 : TW * h + TW],
                            )
                            # fence-bridge touches (see fdum above): tiny per
                            # (window, head) reads/writes in fdum's window-w
                            # region, so proj(w)'s fence read overlaps exactly
                            # its own window's touches and no other's (the RAW
                            # dep is instruction-granular: an 8-col read of the
                            # mul/shift outputs carries the full dependency)
                            nc.vector.tensor_copy(
                                fdum[0:1, ds(TW * w + 16 * h, 8)],
                                att2[0:1, w, ds(TW * h, 8)],
                            )
                            nc.vector.tensor_copy(
                                fdum[DK : DK + 1, ds(TW * w + 16 * h, 8)],
                                att2[DK : DK + 1, w, ds(TW * h, 8)],
                            )

                    # single-dim split view (baseline-proven pattern): col
                    # index c = 32h + r merges (h, r) naturally in memory
                    a2v = att2.rearrange("p w (c t) -> p w c t", t=TT)

                    def proj(fw):
                        def go():
                            yp = psa.tile([P, 2 * TW], F32, tag="s", bufs=2, name="yp")
                            if fw == 0:
                                # fence 1: moving-operand read of fdum's
                                # window-w region (written by the DVE touches
                                # after this window's muls and shift DMAs)
                                # stalls the PE dispatch.
                                nc.tensor.matmul(
                                    yp[0:1, 0:DK],
                                    zer[:, 0:1],
                                    fdum[:, ds(TW * w, DK)],
                                    start=True,
                                    stop=True,
                                )
                                # fence 2: an explicit LDWEIGHTS reading att2
                                # carries the RAW wait on the weight-load
                                # itself — weight-loads execute in order, so
                                # the PE reorder window cannot prefetch the
                                # proj lhsT past this waiting load.
                                nc.tensor.ldweights(
                                    weights=a2v[:, w, :, 0:1]
                                )
                            for m in range(EC):
                                nc.tensor.matmul(
                                    yp[:, 0:TW],
                                    a2v[:, w, :, 2 * m : 2 * m + 1],
                                    wp_sb[:, m, ds(TW * fw, TW)],
                                    start=(m == 0),
                                    stop=(m == EC - 1),
                                )
                            ysb = ypool.tile([P, TW], F32, name="ysb")
                            nc.vector.scalar_tensor_tensor(
                                out=ysb,
                                in0=yp[:, 0:TW],
                                scalar=1.0,
                                in1=bp_b[:, ds(TW * fw, TW)],
                                op0=ALU.mult,
                                op1=ALU.add,
                            )
                            # simple 2D store in (w, h, r) row order; the host
                            # reorders to (h, w, r)
                            engs[(2 * w + fw) % 2].dma_start(
                                out=y[ds(P * w, P), ds(TW * fw, TW)],
                                in_=ysb,
                            )

                        return go

                    return [drain, recip, norm2, proj(0), proj(1)]

                def emit_pv(pvs, ess, jc, njc, trim):
                    # trim: on diagonal blocks, q-cols < 128*qq are fully
                    # masked (es there is stale/garbage) — skip them. Safe:
                    # every pv element's FIRST writer is the full-width jc==0
                    # block (qq<=0 there), so start-flag coverage is complete.
                    wj, jj = divmod(jc, JPW)
                    for es, p in ess:
                        for sub in range(2):
                            h = 2 * p + sub
                            nc.tensor.matmul(
                                pvs[h][0 : DK + 1, trim:TW],
                                vsbw[wj][:, jj, ds((DK + 1) * h, DK + 1)],
                                es[:, TW * sub + trim : TW * (sub + 1)],
                                start=(jc == 0),
                                stop=(jc == njc - 1),
                            )

                def attn_phase(w, pvs, ps):
                    # scores -> exp -> mask -> pv for head-pairs in ps, over
                    # all causally-live j-blocks of window w
                    njc = JPW * (w + 1)
                    pend = []
                    for jc in range(njc):
                        qq = jc - JPW * w  # >=0 on causal-diagonal blocks
                        trim = P * qq if qq > 0 else 0
                        ess = []
                        for p in ps:
                            st = psa.tile([P, 2 * TW], F32, tag="s", bufs=2, name="st")
                            for sub in range(2):
                                nc.tensor.matmul(
                                    st[:, TW * sub + trim : TW * (sub + 1)],
                                    kTw[jc // JPW][
                                        ds(DK * sub, DK), p, ds(P * (jc % JPW), P)
                                    ],
                                    qTw[w][ds(DK * sub, DK), p, trim:TW],
                                    start=True,
                                    stop=True,
                                )
                            es = epool.tile([P, 2 * TW], BF16, name="es")
                            if trim:
                                # strided view skips the fully-masked q-cols of
                                # both subs in ONE ACTIVATE (ACT is co-critical)
                                nc.scalar.activation(
                                    es.rearrange("p (s t) -> p s t", s=2)[
                                        :, :, trim:TW
                                    ],
                                    st.rearrange("p (s t) -> p s t", s=2)[
                                        :, :, trim:TW
                                    ],
                                    AF.Exp,
                                    scale=1.0 / 8.0,
                                )
                            else:
                                nc.scalar.activation(es, st, AF.Exp, scale=1.0 / 8.0)
                            if qq >= 0:
                                mw = P * (qq + 1)  # mask strip end (rest keep-all)
                                for sub in range(2):
                                    nc.vector.tensor_mul(
                                        es[:, TW * sub + trim : TW * sub + mw],
                                        es[:, TW * sub + trim : TW * sub + mw],
                                        masks[qq][:, trim:mw],
                                    )
                            ess.append((es, p))
                        pend.append((ess, jc, trim))
                        pump()
                        if len(pend) > 2:
                            e0 = pend.pop(0)
                            emit_pv(pvs, e0[0], e0[1], njc, e0[2])
                            pump()
                    for e0 in pend:
                        emit_pv(pvs, e0[0], e0[1], njc, e0[2])
                        pump()

                for w in range(W):
                    # queue next window's QKV groups (w0's were interleaved above)
                    if 0 < w < W - 1:
                        queue_qkv(w + 1)
                    elif w == 0:
                        for i0 in range(JPW):
                            side_q.append(lambda i0=i0: emit_v_one(1, i0))

                    pvs = [
                        psa.tile([P, TW], F32, tag=f"pv{h}", bufs=1, name=f"pv{h}")
                        for h in range(HPC)
                    ]
                    if w + 1 < W:
                        attn_phase(w, pvs, [0, 1])
                        # drain pv banks NOW (before the next window's pv ring
                        # allocation — the ring's WAR tracking only sees
                        # readers already emitted); the rest is deferred
                        chain = make_norm_chain(w, pvs, [0, 1, 2, 3])
                        chain[0]()
                        side_q.extend(chain[1:3])
                        projq.extend(chain[3:])
                    else:
                        # final window, split by head pair: heads 0/1's norm
                        # chain flows on DVE/DMA/gpsimd while heads 2/3's
                        # blocks keep the PE busy; the remaining h2/h3 chain
                        # then hides under windows 0-2's deferred projections.
                        attn_phase(w, pvs, [0])
                        chainA = make_norm_chain(w, pvs, [0, 1])
                        chainA[0]()
                        side_q.extend(chainA[1:3])
                        attn_phase(w, pvs, [1])
                        chainB = make_norm_chain(
                            w, pvs, [2, 3], drain_engs=[nc.vector, nc.scalar]
                        )
                        chainB[0]()
                        chainB[1]()
                        chainB[2]()
                        for pj in projq:
                            pj()
                        chainB[3]()
                        chainB[4]()
    nc.compile()
    return nc


_CACHE = {}
LAST_RESULT = None


def _get_nc(T=2048):
    key = ("nc", T)
    if key not in _CACHE:
        _CACHE[key] = build_nc(T=T)
    return _CACHE[key]


def make_in_maps(x, w_qkv, w_proj, b_proj):
    B, T, _E = x.shape
    in_maps = []
    wpTh = np.ascontiguousarray(w_proj.T.astype(BF16NP))
    bph = np.ascontiguousarray(b_proj.reshape(1, E).astype(BF16NP))
    xTs = [np.ascontiguousarray(x[b].T.astype(BF16NP)) for b in range(B)]
    for c in range(8):
        b, g = divmod(c, 4)
        r0 = HPC * DK * g  # 256*g
        sl = slice(r0, r0 + HPC * DK)
        wqkvTh = np.ascontiguousarray(
            np.concatenate(
                [
                    w_qkv[sl, :].T,
                    w_qkv[E:][sl, :].T,
                    w_qkv[2 * E :][sl, :].T,
                ],
                axis=1,
            ).astype(BF16NP)
        )
        in_maps.append(
            {
                "xT": xTs[b],
                "wqkvT": wqkvTh,
                "wpT": wpTh,
                "bp": bph,
            }
        )
    return in_maps


def kernel(x, w_qkv, w_proj, b_proj):
    global LAST_RESULT
    from concourse.bass_utils import run_bass_kernel_spmd

    x = np.asarray(x, dtype=np.float32)
    w_qkv = np.asarray(w_qkv, dtype=np.float32)
    w_proj = np.asarray(w_proj, dtype=np.float32)
    b_proj = np.asarray(b_proj, dtype=np.float32)
    B, T, _E = x.shape

    nc = _get_nc(T=T)
    in_maps = make_in_maps(x, w_qkv, w_proj, b_proj)
    res = run_bass_kernel_spmd(nc, in_maps, core_ids=list(range(8)))
    LAST_RESULT = res

    out = np.empty((B, T, E), dtype=np.float32)
    rows = HPC * ((T * DK) // E)  # 512 rows per core
    W = T // 512
    RW = ((T * DK) // E) // W
    for c in range(8):
        b, g = divmod(c, 4)
        yc = res.results[c]["y"].reshape(W, HPC, RW, E).transpose(1, 0, 2, 3)
        out[b, rows * g : rows * (g + 1), :] = yc.reshape(rows, E)
    return out

